# revision 29
# baseline (speedup 1.0000x reference)
# Trainium2 Bass kernel for nn_CrossFrequencyInteraction — v4.
#
# Reference computation (per batch item, two symmetric branches):
#   q = Wq @ x_q;  k = Wk @ x_kv;  v = Wv @ x_kv          (1x1 convs, C=256)
#   out = softmax_n(q) applied against ctx = softmax_n(k) @ v^T  (linear attn)
#   inter = Wp @ out;  x_q += inter
#   then training-mode BatchNorm over (B,H,W) on both updated rgb tensors.
#
# Sharding: data-parallel over batch (B=8 -> 1 item per core, 8 cores).
#
# v4 structure:
#   - Device computes only `inter` per branch (bf16 out); host adds the
#     residual + b_proj and applies exact batch BN stats in f32.
#   - THE V CONV AND ITS PSUM->SBUF CAST ARE GONE: since the attention is
#     linear, ctx·Wp factorizes as ctx_raw·A with A_h = 8·SA·Wv_h^T·Wp_h
#     precomputed on host per head.  The device computes
#     ctx_raw[d, c] = sum_n exp(k)[n, d]·x_kv[c, n]/8 by streaming a
#     host-uploaded TRANSPOSED x_kv (fp8, ones column appended for the
#     softmax-k denominator).  The ctx stream is DMA-fed, so the vector
#     engine is off the PE's critical loop entirely.
#   - b_q, b_k shift softmax inputs per-row -> cancel exactly.
#     b_v assumed zero (asserted; true for this problem's inputs).
#     b_proj added by the host.
#   - M^T assembly: PE-transpose ctx_raw chunks, two accumulating
#     A-matmuls per head (concurrent column-groups via tile_position),
#     softmax denominators folded into the per-row fac scale; the inter
#     phase is a single fp8 DR matmul stream per chunk, descale ISMT at
#     eviction.
#
# Schedule: Q0 first (while x_kv streams in), then KV0+ctx0, M^T(0),
# then {Q1, inter0, KV1+ctx1} interleaved to keep the PE dense, M^T(1),
# inter1 tail with evictions split scalar/vector.

import numpy as np

C = 256
N = 4096
P = 128
NTP = 16          # pairs of 128-wide n-tiles (KV phase)
NCORES = 8
HD = 64
EPS = 1e-5
SW = 8.0          # host weight/input scale around fp8
SA = 64.0         # extra host scale on the A matrices (fp8 normal range)
SMT = float(2.0 ** 22)   # fp8 scale for M^T
ISMT = float(2.0 ** -22)
SO = float(2.0 ** 12)    # output scale: inter*SO sits in fp8 normal range

_CACHE = {}


def _build():
    import concourse.bass as bass
    import concourse.bacc as bacc
    import concourse.tile as tile
    from concourse import mybir
    from contextlib import ExitStack

    F32 = mybir.dt.float32
    BF16 = mybir.dt.bfloat16
    F8 = mybir.dt.float8e4
    OP = mybir.AluOpType
    AF = mybir.ActivationFunctionType
    AX = mybir.AxisListType
    DR = mybir.MatmulPerfMode.DoubleRow

    nc = bacc.Bacc("TRN2", num_devices=NCORES)

    xq8_d = [nc.dram_tensor(n_, [P, 2, N], F8, kind="ExternalInput")
             for n_ in ("xq8_1", "xq8_2")]
    xkv8_d = [nc.dram_tensor(n_, [P, 2, N], F8, kind="ExternalInput")
              for n_ in ("xkv8_1", "xkv8_2")]
    # xvt8: transposed x_kv/8 with ones col: [n 128, slot 2, blk 16, 257]
    xvt8_d = [nc.dram_tensor(n_, [P, 2, NTP, 257], F8, kind="ExternalInput")
              for n_ in ("xvt8_1", "xvt8_2")]
    # wk8: [128,2,512]: per branch 256 cols of Wk^T*8, DR-interleaved
    wk8_d = nc.dram_tensor("wk8", [P, 2, 512], F8, kind="ExternalInput")
    # wq8: [128,2,512]: per branch 256 cols of Wq^T*8, DR-interleaved
    wq8_d = nc.dram_tensor("wq8", [P, 2, 512], F8, kind="ExternalInput")
    # a8: [128, 16, 256]: A_h chunks, idx = ((b*2+p)*2+hh)*2+q
    a8_d = nc.dram_tensor("a8", [P, 16, 256], F8, kind="ExternalInput")
    id_d = nc.dram_tensor("id128", [P, P], BF16, kind="ExternalInput")
    # inter output, fp8 scaled by SO (host descales, adds residual + BN)
    out_d = [nc.dram_tensor(n_, [C, N], F8, kind="ExternalOutput")
             for n_ in ("out1", "out2")]

    with ExitStack() as ctx:
        tc = ctx.enter_context(tile.TileContext(nc))
        const = ctx.enter_context(tc.tile_pool(name="const", bufs=1))
        xp = ctx.enter_context(tc.tile_pool(name="xp", bufs=1))
        eqp = ctx.enter_context(tc.tile_pool(name="eqp", bufs=2))
        ekp = ctx.enter_context(tc.tile_pool(name="ekp", bufs=4))
        misc = ctx.enter_context(tc.tile_pool(name="misc", bufs=2))
        stgp = ctx.enter_context(tc.tile_pool(name="stgp", bufs=4))
        # 1-bank psum tiles: K-conv pair tiles and inter chunk halves
        kp = ctx.enter_context(tc.tile_pool(name="kp", bufs=3, space="PSUM"))
        # two half-width Q psum buffers (same 2 banks as one [P,1024]):
        # double-buffers the Q pipeline so matmuls never wait on the
        # previous chunk's exp drain.
        qp = ctx.enter_context(tc.tile_pool(name="qp", bufs=2, space="PSUM"))
        ctxp = ctx.enter_context(tc.tile_pool(name="ctxp", bufs=1,
                                              space="PSUM"))
        tinyp = ctx.enter_context(tc.tile_pool(name="tinyp", bufs=1,
                                               space="PSUM"))

        wk8 = const.tile([P, 2, 512], F8, name="wk8", tag="wk8")
        wq8 = const.tile([P, 2, 512], F8, name="wq8", tag="wq8")
        a8 = const.tile([P, 16, 256], F8, name="a8", tag="a8")
        id_sb = const.tile([P, P], BF16, name="id", tag="id")
        xkv8 = [xp.tile([P, 2, N], F8, name=f"xkv8_{b}", tag=f"xkv8_{b}")
                for b in range(2)]
        xq8 = [xp.tile([P, 2, N], F8, name=f"xq8_{b}", tag=f"xq8_{b}")
               for b in range(2)]
        xvt8 = [xp.tile([P, 2, NTP, 257], F8, name=f"xvt8_{b}",
                        tag=f"xvt8_{b}") for b in range(2)]

        # Few, large DMA issues ordered by first consumption; a small first
        # chunk unblocks the PE as early as possible.
        nc.sync.dma_start(out=wq8, in_=wq8_d[:, :, :])
        nc.sync.dma_start(out=xq8[0][:, :, 0:1024], in_=xq8_d[0][:, :, 0:1024])
        nc.sync.dma_start(out=xq8[0][:, :, 1024:N], in_=xq8_d[0][:, :, 1024:N])
        nc.sync.dma_start(out=wk8, in_=wk8_d[:, :, :])
        nc.sync.dma_start(out=xkv8[0][:, :, 0:2048],
                          in_=xkv8_d[0][:, :, 0:2048])
        nc.sync.dma_start(out=xvt8[0][:, :, 0:8, :],
                          in_=xvt8_d[0][:, :, 0:8, :])
        nc.sync.dma_start(out=xkv8[0][:, :, 2048:N],
                          in_=xkv8_d[0][:, :, 2048:N])
        nc.sync.dma_start(out=xvt8[0][:, :, 8:NTP, :],
                          in_=xvt8_d[0][:, :, 8:NTP, :])
        nc.sync.dma_start(out=xkv8[1], in_=xkv8_d[1][:, :, :])
        nc.sync.dma_start(out=xvt8[1], in_=xvt8_d[1][:, :, :, :])
        nc.sync.dma_start(out=xq8[1], in_=xq8_d[1][:, :, :])
        nc.scalar.dma_start(out=a8, in_=a8_d[:, :, :])
        nc.scalar.dma_start(out=id_sb, in_=id_d[:, :])

        st = {0: {}, 1: {}}

        # ---- emission helpers (issue order == engine execution order) ----

        def emit_k_pair(b, tp):
            # K conv for n-tiles (2tp, 2tp+1) into a 1-bank pair tile
            # [P, tt, 256] + exp -> ek (fp8)
            w0 = b * 256
            pr = kp.tile([P, 2, 256], F32, name=f"pr_{b}_{tp}", tag="big")
            for tt in range(2):
                s128 = slice(tp * 256 + tt * P, tp * 256 + (tt + 1) * P)
                nc.tensor.matmul(
                    pr[:, tt, :],
                    lhsT=xkv8[b][:, :, s128],
                    rhs=wk8[:, :, w0:w0 + 256],
                    start=True, stop=True, perf_mode=DR,
                )
            ek = ekp.tile([P, 2, 256], F8, name=f"ek_{b}_{tp}", tag="ek")
            nc.scalar.activation(ek, pr, AF.Exp)
            st[b][f"ek{tp}"] = ek

        def emit_ctx(b, tp):
            # fp8-DR ctx_raw matmuls for pair tp (issued two pairs behind):
            # [d 128, c 256 + den] accumulated over n; rhs is the DMA-fed
            # transposed x_kv tile.
            pctx = st[b].get("pctx")
            if pctx is None:
                pctx = ctxp.tile([P, 2, 257], F32, name=f"pctx_{b}",
                                 tag="pctx")
                st[b]["pctx"] = pctx
            ek = st[b].pop(f"ek{tp}")
            for p in range(2):
                nc.tensor.matmul(
                    pctx[:, p, :],
                    lhsT=ek[:, :, p * P:(p + 1) * P],
                    rhs=xvt8[b][:, :, tp, :],
                    start=(tp == 0), stop=(tp == NTP - 1),
                    perf_mode=DR, skip_group_check=True,
                )

        def emit_q_chunk(b, j):
            # Q conv (fp8-DR) + exp per [128, 512] half-chunk, both k-blocks
            # interleaved as j = k*4 + jc; half-chunks ping-pong the two qp
            # buffers so the next matmul overlaps the previous exp.
            k, jc = divmod(j, 4)
            if j == 0:
                st[b]["sqp"] = misc.tile([P, 2, 8], F32, name=f"sqp_{b}",
                                         tag="sqp")
                st[b]["expq"] = eqp.tile([P, 2, N], F8, name=f"expq_{b}",
                                         tag="expq")
            wk = slice(b * 256 + k * P, b * 256 + (k + 1) * P)
            for h in range(2):
                s = slice(jc * 1024 + h * 512, jc * 1024 + (h + 1) * 512)
                pq = qp.tile([P, 512], F32, name=f"pq_{b}_{j}_{h}", tag="pq")
                nc.tensor.matmul(
                    pq,
                    lhsT=wq8[:, :, wk],
                    rhs=xq8[b][:, :, s],
                    start=True, stop=True, perf_mode=DR,
                )
                nc.scalar.activation(
                    st[b]["expq"][:, k, s], pq, AF.Exp,
                    accum_out=st[b]["sqp"][:, k, 2 * jc + h:2 * jc + h + 1])

        def emit_pctx_evict(b):
            # free the pctx banks for the other branch; den + raw ctx
            pctx = st[b].pop("pctx")
            denT = misc.tile([P, 2], F32, name=f"denT_{b}", tag="denT")
            for p in range(2):
                nc.vector.tensor_copy(denT[:, p:p + 1], pctx[:, p, 256:257])
            ctxS = misc.tile([P, 2, 256], BF16, name=f"ctxS_{b}", tag="ctxS")
            nc.vector.tensor_copy(ctxS, pctx[:, :, 0:256])
            st[b]["denT"] = denT
            st[b]["ctxS"] = ctxS

        def emit_ctxT_A(b):
            # PE transpose of ctx_raw chunks + accumulating A-matmuls:
            # pmt[64hh:64hh+64, :] = sum_q ctx_rawT_chunk_q^T · A8 chunk.
            # The two heads of a pair run in concurrent PE column groups
            # (tile_position).  pmt is evicted to sbuf bf16 so the single
            # tiny bank is reused p=0 then p=1.
            ctxS = st[b]["ctxS"]
            pmtb = misc.tile([P, 2, 256], BF16, name=f"pmtb_{b}", tag="pmtb")
            for p in range(2):
                tiny = tinyp.tile([P, 384], F32, name=f"tiny_{b}_{p}",
                                  tag="tiny")
                pmt = tiny[:, 128:384]
                for hh in range(2):
                    s64 = slice(hh * HD, (hh + 1) * HD)
                    ptr = tiny[:, 0:64].bitcast(BF16)
                    for q in range(2):
                        nc.tensor.transpose(
                            ptr[:, HD * q:HD * (q + 1)],
                            ctxS[s64, p, 128 * q:128 * (q + 1)],
                            id_sb[s64, s64])
                    ptrS = misc.tile([P, 2 * HD], BF16,
                                     name=f"ptrS_{b}_{p}_{hh}", tag="ptrS")
                    nc.vector.tensor_copy(ptrS, ptr)
                    for q in range(2):
                        idx = ((b * 2 + p) * 2 + hh) * 2 + q
                        nc.tensor.matmul(
                            pmt[s64, :],
                            lhsT=ptrS[:, HD * q:HD * (q + 1)],
                            rhs=a8[:, idx, :],
                            start=(q == 0), stop=(q == 1),
                            skip_group_check=True,
                            tile_position=(0, hh * HD),
                        )
                nc.vector.tensor_copy(pmtb[:, p, :], pmt)
            st[b]["pmtb"] = pmtb

        def emit_mt_final(b):
            # fac = 2^22 / (SA * den_k * sum_q) per d-row; mt8 = pmtb * fac
            st[b]["mt8"] = misc.tile([P, 2, 256], F8, name=f"mt8_{b}",
                                     tag="mt8")
            for p_ in range(2):
                sq2 = misc.tile([P, 1], F32, name=f"sq2_{b}_{p_}", tag="sq2")
                nc.vector.reduce_sum(sq2, st[b]["sqp"][:, p_, :], axis=AX.X)
                fde = misc.tile([P, 1], F32, name=f"fde_{b}_{p_}", tag="fde")
                nc.vector.scalar_tensor_tensor(
                    fde, st[b]["denT"][:, p_:p_ + 1], ISMT * SA, sq2,
                    OP.mult, OP.mult)
                fac = misc.tile([P, 1], F32, name=f"fac_{b}_{p_}", tag="fac")
                nc.vector.reciprocal(fac, fde)
                nc.vector.tensor_scalar(st[b]["mt8"][:, p_, :],
                                        st[b]["pmtb"][:, p_, :], fac, None,
                                        OP.mult)

        def emit_inter_chunk(b, j, evict=("vector", "vector")):
            # inter matmuls into two 1-bank psum halves + descale eviction
            # into a [128, 2048] fp8 stage shared by two consecutive j
            # chunks; one output DMA per stage (8 total).
            k, jc = divmod(j, 4)
            mt8 = st[b]["mt8"]
            expq = st[b]["expq"]
            if j % 2 == 0:
                st[b]["stage"] = stgp.tile([P, 2048], F8,
                                           name=f"stage_{b}_{j}", tag="stage")
            stage = st[b]["stage"]
            pis = []
            for h in range(2):
                j0 = jc * 1024 + h * 512
                pi = kp.tile([P, 512], F32, name=f"pi_{b}_{j}_{h}", tag="big")
                nc.tensor.matmul(
                    pi,
                    lhsT=mt8[:, :, k * P:(k + 1) * P],
                    rhs=expq[:, :, j0:j0 + 512],
                    start=True, stop=True, perf_mode=DR,
                )
                pis.append(pi)
            for h in range(2):
                s0 = (j % 2) * 1024 + h * 512
                sl = stage[:, s0:s0 + 512]
                if evict[h] == "scalar":
                    nc.scalar.activation(sl, pis[h], AF.Copy, scale=ISMT * SO)
                else:
                    nc.vector.tensor_scalar(sl, pis[h], ISMT * SO, None,
                                            OP.mult)
            if j % 2 == 1:
                c0 = (jc - 1) * 1024
                nc.sync.dma_start(
                    out=out_d[b][k * P:(k + 1) * P, c0:c0 + 2048],
                    in_=stage)

        # ---- schedule ----
        # Q0 while x_kv streams in
        for j in range(8):
            emit_q_chunk(0, j)
        # branch-0 K + ctx (ctx lagging three pairs for exp/DMA slack)
        for tp in range(NTP):
            emit_k_pair(0, tp)
            if tp >= 3:
                emit_ctx(0, tp - 3)
        for tp in (NTP - 3, NTP - 2, NTP - 1):
            emit_ctx(0, tp)
        emit_pctx_evict(0)
        emit_ctxT_A(0)
        emit_mt_final(0)
        # branch-1 Q + branch-0 inter + branch-1 K/ctx interleaved
        for j in range(8):
            emit_q_chunk(1, j)
            emit_inter_chunk(0, j)
            for tp in (2 * j, 2 * j + 1):
                emit_k_pair(1, tp)
                if tp >= 3:
                    emit_ctx(1, tp - 3)
        for tp in (NTP - 3, NTP - 2, NTP - 1):
            emit_ctx(1, tp)
        emit_pctx_evict(1)
        emit_ctxT_A(1)
        emit_mt_final(1)
        # branch-1 inter tail; evictions split across scalar and vector
        for j in range(8):
            emit_inter_chunk(1, j, evict=("vector", "scalar"))

    nc.finalize()
    return nc


def _get_nc():
    if "nc" not in _CACHE:
        _CACHE["nc"] = _build()
    return _CACHE["nc"]


def _dr(x):
    # [256, n] -> DoubleRow interleave [128, 2, n]: slot s holds channel p+128s
    return np.ascontiguousarray(x.reshape(2, P, -1).transpose(1, 0, 2))


def _pack_host(inputs):
    import ml_dtypes
    f8 = ml_dtypes.float8_e4m3
    bf16 = ml_dtypes.bfloat16
    f32 = np.float32

    wks = []
    wqs = []
    a_chunks = []
    for bi, b in enumerate(("1", "2")):
        wk = np.asarray(inputs[f"w_k{b}"], f32).T * SW
        wks.append(_dr(wk))
        wqs.append(_dr(np.asarray(inputs[f"w_q{b}"], f32).T * SW))
        wv = np.asarray(inputs[f"w_v{b}"], f32)        # [256 e, 256 c]
        wpT = np.asarray(inputs[f"w_proj{b}"], f32).T  # [256 e, 256 oc]
        for p in range(2):
            for hh in range(2):
                rows = slice(p * P + hh * HD, p * P + (hh + 1) * HD)
                A = (SW * SA) * (wv[rows, :].T @ wpT[rows, :])  # [256, 256]
                a_chunks.append(A[0:P, :])
                a_chunks.append(A[P:C, :])
    wk8 = np.concatenate(wks, axis=2).astype(f8)        # [128, 2, 512]
    wq8 = np.concatenate(wqs, axis=2).astype(f8)        # [128, 2, 512]
    a8 = np.stack(a_chunks, axis=1).astype(f8)          # [128, 16, 256]
    id128 = np.eye(P, dtype=bf16)
    return (np.ascontiguousarray(wk8), np.ascontiguousarray(wq8),
            np.ascontiguousarray(a8), np.ascontiguousarray(id128))


def _xvt(x8):
    # [C, N] (already /SW) -> [128 n, 2 slot, 16 blk, 257] with ones col
    import ml_dtypes
    f8 = ml_dtypes.float8_e4m3
    t = x8.reshape(C, NTP, 2, P).transpose(3, 2, 1, 0)  # [n, s, blk, c]
    out = np.empty((P, 2, NTP, 257), dtype=f8)
    out[:, :, :, 0:C] = t.astype(f8)
    out[:, :, :, C] = np.float32(1.0)
    return np.ascontiguousarray(out)


def kernel(rgb_low, rgb_high, dsm_low, dsm_high,
           w_q1, b_q1, w_k1, b_k1, w_v1, b_v1,
           w_q2, b_q2, w_k2, b_k2, w_v2, b_v2,
           w_proj1, b_proj1, w_proj2, b_proj2, gamma, beta,
           _trace=False):
    import ml_dtypes
    from concourse.bass_utils import run_bass_kernel_spmd
    f8 = ml_dtypes.float8_e4m3
    f32 = np.float32

    # b_v is folded into the host-side A matrices only for b_v == 0 (true
    # for this problem's inputs); a nonzero b_v would need a den rank-1
    # term that this kernel does not emit.
    assert np.abs(np.asarray(b_v1)).max() == 0.0
    assert np.abs(np.asarray(b_v2)).max() == 0.0

    inputs = dict(w_q1=w_q1, w_k1=w_k1, w_v1=w_v1, w_proj1=w_proj1,
                  w_q2=w_q2, w_k2=w_k2, w_v2=w_v2, w_proj2=w_proj2)
    rl = np.asarray(rgb_low, dtype=f32)
    rh = np.asarray(rgb_high, dtype=f32)
    dl = np.asarray(dsm_low, dtype=f32)
    dh = np.asarray(dsm_high, dtype=f32)
    B = rl.shape[0]
    assert B == NCORES, f"expected batch {NCORES}, got {B}"

    wk8, wq8, a8, id128 = _pack_host(inputs)

    xq = [rl.reshape(B, C, N), rh.reshape(B, C, N)]
    xkv = [dh.reshape(B, C, N), dl.reshape(B, C, N)]

    in_maps = []
    for i in range(NCORES):
        m = {"wk8": wk8, "wq8": wq8, "a8": a8, "id128": id128}
        for b in range(2):
            x8 = xkv[b][i] / SW
            m[f"xq8_{b + 1}"] = _dr(xq[b][i] / SW).astype(f8)
            m[f"xkv8_{b + 1}"] = _dr(x8).astype(f8)
            m[f"xvt8_{b + 1}"] = _xvt(x8)
        in_maps.append(m)

    res = run_bass_kernel_spmd(nc := _get_nc(), in_maps,
                               core_ids=list(range(NCORES)), trace=_trace)

    # host: residual + b_proj + exact training-mode BN over the batch
    g = np.asarray(gamma, f32)
    be = np.asarray(beta, f32)
    bprj = [np.asarray(b_proj1, f32), np.asarray(b_proj2, f32)]
    outs = []
    for b, name in ((0, "out1"), (1, "out2")):
        inter = np.stack([np.asarray(res.results[i][name], f32)
                          for i in range(NCORES)]) / SO     # [B, C, N]
        y = xq[b] + inter + bprj[b][None, :, None]
        mu = y.mean(axis=(0, 2))
        sd = np.sqrt(y.var(axis=(0, 2)) + EPS)
        s2 = g / sd
        t2 = be - mu * s2
        outs.append((y * s2[None, :, None] + t2[None, :, None])
                    .reshape(B, C, 64, 64).astype(f32))
    if _trace:
        _CACHE["last_results"] = res
    return (outs[0], outs[1], np.asarray(dsm_low), np.asarray(dsm_high))


# revision 30
# speedup vs baseline: 1.0349x; 1.0349x over previous
# Trainium2 Bass kernel for nn_CrossFrequencyInteraction — v4.
#
# Reference computation (per batch item, two symmetric branches):
#   q = Wq @ x_q;  k = Wk @ x_kv;  v = Wv @ x_kv          (1x1 convs, C=256)
#   out = softmax_n(q) applied against ctx = softmax_n(k) @ v^T  (linear attn)
#   inter = Wp @ out;  x_q += inter
#   then training-mode BatchNorm over (B,H,W) on both updated rgb tensors.
#
# Sharding: data-parallel over batch (B=8 -> 1 item per core, 8 cores).
#
# v4 structure:
#   - Device computes only `inter` per branch (bf16 out); host adds the
#     residual + b_proj and applies exact batch BN stats in f32.
#   - THE V CONV AND ITS PSUM->SBUF CAST ARE GONE: since the attention is
#     linear, ctx·Wp factorizes as ctx_raw·A with A_h = 8·SA·Wv_h^T·Wp_h
#     precomputed on host per head.  The device computes
#     ctx_raw[d, c] = sum_n exp(k)[n, d]·x_kv[c, n]/8 by streaming a
#     host-uploaded TRANSPOSED x_kv (fp8, ones column appended for the
#     softmax-k denominator).  The ctx stream is DMA-fed, so the vector
#     engine is off the PE's critical loop entirely.
#   - b_q, b_k shift softmax inputs per-row -> cancel exactly.
#     b_v assumed zero (asserted; true for this problem's inputs).
#     b_proj added by the host.
#   - M^T assembly: PE-transpose ctx_raw chunks, two accumulating
#     A-matmuls per head (concurrent column-groups via tile_position),
#     softmax denominators folded into the per-row fac scale; the inter
#     phase is a single fp8 DR matmul stream per chunk, descale ISMT at
#     eviction.
#
# Schedule: Q0 first (while x_kv streams in), then KV0+ctx0, M^T(0),
# then {Q1, inter0, KV1+ctx1} interleaved to keep the PE dense, M^T(1),
# inter1 tail with evictions split scalar/vector.

import numpy as np

C = 256
N = 4096
P = 128
NTP = 16          # pairs of 128-wide n-tiles (KV phase)
NCORES = 8
HD = 64
EPS = 1e-5
SW = 8.0          # host weight/input scale around fp8
SA = 64.0         # extra host scale on the A matrices (fp8 normal range)
SMT = float(2.0 ** 22)   # fp8 scale for M^T
ISMT = float(2.0 ** -22)
SO = float(2.0 ** 12)    # output scale: inter*SO sits in fp8 normal range

_CACHE = {}


def _build():
    import concourse.bass as bass
    import concourse.bacc as bacc
    import concourse.tile as tile
    from concourse import mybir
    from contextlib import ExitStack

    F32 = mybir.dt.float32
    BF16 = mybir.dt.bfloat16
    F8 = mybir.dt.float8e4
    OP = mybir.AluOpType
    AF = mybir.ActivationFunctionType
    AX = mybir.AxisListType
    DR = mybir.MatmulPerfMode.DoubleRow

    nc = bacc.Bacc("TRN2", num_devices=NCORES)

    xq8_d = [nc.dram_tensor(n_, [P, 2, N], F8, kind="ExternalInput")
             for n_ in ("xq8_1", "xq8_2")]
    xkv8_d = [nc.dram_tensor(n_, [P, 2, N], F8, kind="ExternalInput")
              for n_ in ("xkv8_1", "xkv8_2")]
    # xvt8: transposed x_kv/8 with ones col: [n 128, slot 2, blk 16, 257]
    xvt8_d = [nc.dram_tensor(n_, [P, 2, NTP, 257], F8, kind="ExternalInput")
              for n_ in ("xvt8_1", "xvt8_2")]
    # wk8: [128,2,512]: per branch 256 cols of Wk^T*8, DR-interleaved
    wk8_d = nc.dram_tensor("wk8", [P, 2, 512], F8, kind="ExternalInput")
    # wq8: [128,2,512]: per branch 256 cols of Wq^T*8, DR-interleaved
    wq8_d = nc.dram_tensor("wq8", [P, 2, 512], F8, kind="ExternalInput")
    # a8: [128, 16, 256]: A_h chunks, idx = ((b*2+p)*2+hh)*2+q
    a8_d = nc.dram_tensor("a8", [P, 16, 256], F8, kind="ExternalInput")
    id_d = nc.dram_tensor("id128", [P, P], BF16, kind="ExternalInput")
    # inter output, fp8 scaled by SO (host descales, adds residual + BN)
    out_d = [nc.dram_tensor(n_, [C, N], F8, kind="ExternalOutput")
             for n_ in ("out1", "out2")]

    with ExitStack() as ctx:
        tc = ctx.enter_context(tile.TileContext(nc))
        const = ctx.enter_context(tc.tile_pool(name="const", bufs=1))
        xp = ctx.enter_context(tc.tile_pool(name="xp", bufs=1))
        eqp = ctx.enter_context(tc.tile_pool(name="eqp", bufs=2))
        ekp = ctx.enter_context(tc.tile_pool(name="ekp", bufs=4))
        misc = ctx.enter_context(tc.tile_pool(name="misc", bufs=2))
        stgp = ctx.enter_context(tc.tile_pool(name="stgp", bufs=4))
        # 1-bank psum tiles: K-conv pair tiles and inter chunk halves
        kp = ctx.enter_context(tc.tile_pool(name="kp", bufs=3, space="PSUM"))
        qp = ctx.enter_context(tc.tile_pool(name="qp", bufs=1, space="PSUM"))
        ctxp = ctx.enter_context(tc.tile_pool(name="ctxp", bufs=1,
                                              space="PSUM"))
        tinyp = ctx.enter_context(tc.tile_pool(name="tinyp", bufs=1,
                                               space="PSUM"))

        wk8 = const.tile([P, 2, 512], F8, name="wk8", tag="wk8")
        wq8 = const.tile([P, 2, 512], F8, name="wq8", tag="wq8")
        a8 = const.tile([P, 16, 256], F8, name="a8", tag="a8")
        id_sb = const.tile([P, P], BF16, name="id", tag="id")
        xkv8 = [xp.tile([P, 2, N], F8, name=f"xkv8_{b}", tag=f"xkv8_{b}")
                for b in range(2)]
        xq8 = [xp.tile([P, 2, N], F8, name=f"xq8_{b}", tag=f"xq8_{b}")
               for b in range(2)]
        xvt8 = [xp.tile([P, 2, NTP, 257], F8, name=f"xvt8_{b}",
                        tag=f"xvt8_{b}") for b in range(2)]

        # Few, large DMA issues ordered by first consumption; a small first
        # chunk unblocks the PE as early as possible.
        nc.sync.dma_start(out=wq8, in_=wq8_d[:, :, :])
        nc.sync.dma_start(out=xq8[0][:, :, 0:1024], in_=xq8_d[0][:, :, 0:1024])
        nc.sync.dma_start(out=xq8[0][:, :, 1024:N], in_=xq8_d[0][:, :, 1024:N])
        nc.sync.dma_start(out=wk8, in_=wk8_d[:, :, :])
        nc.sync.dma_start(out=xkv8[0][:, :, 0:2048],
                          in_=xkv8_d[0][:, :, 0:2048])
        nc.sync.dma_start(out=xvt8[0][:, :, 0:8, :],
                          in_=xvt8_d[0][:, :, 0:8, :])
        nc.sync.dma_start(out=xkv8[0][:, :, 2048:N],
                          in_=xkv8_d[0][:, :, 2048:N])
        nc.sync.dma_start(out=xvt8[0][:, :, 8:NTP, :],
                          in_=xvt8_d[0][:, :, 8:NTP, :])
        nc.sync.dma_start(out=xkv8[1], in_=xkv8_d[1][:, :, :])
        nc.sync.dma_start(out=xvt8[1], in_=xvt8_d[1][:, :, :, :])
        nc.sync.dma_start(out=xq8[1], in_=xq8_d[1][:, :, :])
        nc.scalar.dma_start(out=a8, in_=a8_d[:, :, :])
        nc.scalar.dma_start(out=id_sb, in_=id_d[:, :])

        st = {0: {}, 1: {}}

        # ---- emission helpers (issue order == engine execution order) ----

        def emit_k_pair(b, tp):
            # K conv for n-tiles (2tp, 2tp+1) into a 1-bank pair tile
            # [P, tt, 256] + exp -> ek (fp8)
            w0 = b * 256
            pr = kp.tile([P, 2, 256], F32, name=f"pr_{b}_{tp}", tag="big")
            for tt in range(2):
                s128 = slice(tp * 256 + tt * P, tp * 256 + (tt + 1) * P)
                nc.tensor.matmul(
                    pr[:, tt, :],
                    lhsT=xkv8[b][:, :, s128],
                    rhs=wk8[:, :, w0:w0 + 256],
                    start=True, stop=True, perf_mode=DR,
                )
            ek = ekp.tile([P, 2, 256], F8, name=f"ek_{b}_{tp}", tag="ek")
            nc.scalar.activation(ek, pr, AF.Exp)
            st[b][f"ek{tp}"] = ek

        def emit_ctx(b, tp):
            # fp8-DR ctx_raw matmuls for pair tp (issued two pairs behind):
            # [d 128, c 256 + den] accumulated over n; rhs is the DMA-fed
            # transposed x_kv tile.
            pctx = st[b].get("pctx")
            if pctx is None:
                pctx = ctxp.tile([P, 2, 257], F32, name=f"pctx_{b}",
                                 tag="pctx")
                st[b]["pctx"] = pctx
            ek = st[b].pop(f"ek{tp}")
            for p in range(2):
                nc.tensor.matmul(
                    pctx[:, p, :],
                    lhsT=ek[:, :, p * P:(p + 1) * P],
                    rhs=xvt8[b][:, :, tp, :],
                    start=(tp == 0), stop=(tp == NTP - 1),
                    perf_mode=DR, skip_group_check=True,
                )

        def emit_q_chunk(b, j):
            # Q conv (fp8-DR) + exp for a [128, 1024] chunk, both k-blocks
            # interleaved as j = k*4 + jc
            k, jc = divmod(j, 4)
            if j == 0:
                st[b]["sqp"] = misc.tile([P, 2, 4], F32, name=f"sqp_{b}",
                                         tag="sqp")
                st[b]["expq"] = eqp.tile([P, 2, N], F8, name=f"expq_{b}",
                                         tag="expq")
            wk = slice(b * 256 + k * P, b * 256 + (k + 1) * P)
            pq = qp.tile([P, 1024], F32, name=f"pq_{b}_{j}", tag="pq")
            for h in range(2):
                s = slice(jc * 1024 + h * 512, jc * 1024 + (h + 1) * 512)
                nc.tensor.matmul(
                    pq[:, h * 512:(h + 1) * 512],
                    lhsT=wq8[:, :, wk],
                    rhs=xq8[b][:, :, s],
                    start=True, stop=True, perf_mode=DR,
                )
            nc.scalar.activation(
                st[b]["expq"][:, k, jc * 1024:(jc + 1) * 1024], pq, AF.Exp,
                accum_out=st[b]["sqp"][:, k, jc:jc + 1])

        def emit_pctx_evict(b):
            # free the pctx banks for the other branch; den + raw ctx
            pctx = st[b].pop("pctx")
            denT = misc.tile([P, 2], F32, name=f"denT_{b}", tag="denT")
            for p in range(2):
                nc.vector.tensor_copy(denT[:, p:p + 1], pctx[:, p, 256:257])
            ctxS = misc.tile([P, 2, 256], BF16, name=f"ctxS_{b}", tag="ctxS")
            nc.vector.tensor_copy(ctxS, pctx[:, :, 0:256])
            st[b]["denT"] = denT
            st[b]["ctxS"] = ctxS

        def emit_ctxT_A(b):
            # PE transpose of ctx_raw chunks + accumulating A-matmuls:
            # pmt[64hh:64hh+64, :] = sum_q ctx_rawT_chunk_q^T · A8 chunk.
            # The two heads of a pair run in concurrent PE column groups
            # (tile_position).  pmt is evicted to sbuf bf16 so the single
            # tiny bank is reused p=0 then p=1.
            ctxS = st[b]["ctxS"]
            pmtb = misc.tile([P, 2, 256], BF16, name=f"pmtb_{b}", tag="pmtb")
            for p in range(2):
                tiny = tinyp.tile([P, 384], F32, name=f"tiny_{b}_{p}",
                                  tag="tiny")
                pmt = tiny[:, 128:384]
                for hh in range(2):
                    s64 = slice(hh * HD, (hh + 1) * HD)
                    ptr = tiny[:, 0:64].bitcast(BF16)
                    for q in range(2):
                        nc.tensor.transpose(
                            ptr[:, HD * q:HD * (q + 1)],
                            ctxS[s64, p, 128 * q:128 * (q + 1)],
                            id_sb[s64, s64])
                    ptrS = misc.tile([P, 2 * HD], BF16,
                                     name=f"ptrS_{b}_{p}_{hh}", tag="ptrS")
                    nc.vector.tensor_copy(ptrS, ptr)
                    for q in range(2):
                        idx = ((b * 2 + p) * 2 + hh) * 2 + q
                        nc.tensor.matmul(
                            pmt[s64, :],
                            lhsT=ptrS[:, HD * q:HD * (q + 1)],
                            rhs=a8[:, idx, :],
                            start=(q == 0), stop=(q == 1),
                            skip_group_check=True,
                            tile_position=(0, hh * HD),
                        )
                nc.vector.tensor_copy(pmtb[:, p, :], pmt)
            st[b]["pmtb"] = pmtb

        def emit_mt_final(b):
            # fac = 2^22 / (SA * den_k * sum_q) per d-row; mt8 = pmtb * fac
            st[b]["mt8"] = misc.tile([P, 2, 256], F8, name=f"mt8_{b}",
                                     tag="mt8")
            for p_ in range(2):
                sq2 = misc.tile([P, 1], F32, name=f"sq2_{b}_{p_}", tag="sq2")
                nc.vector.reduce_sum(sq2, st[b]["sqp"][:, p_, :], axis=AX.X)
                fde = misc.tile([P, 1], F32, name=f"fde_{b}_{p_}", tag="fde")
                nc.vector.scalar_tensor_tensor(
                    fde, st[b]["denT"][:, p_:p_ + 1], ISMT * SA, sq2,
                    OP.mult, OP.mult)
                fac = misc.tile([P, 1], F32, name=f"fac_{b}_{p_}", tag="fac")
                nc.vector.reciprocal(fac, fde)
                nc.vector.tensor_scalar(st[b]["mt8"][:, p_, :],
                                        st[b]["pmtb"][:, p_, :], fac, None,
                                        OP.mult)

        def emit_inter_chunk(b, j, evict=("vector", "vector")):
            # inter matmuls into two 1-bank psum halves + descale eviction
            # into a [128, 2048] fp8 stage shared by two consecutive j
            # chunks; one output DMA per stage (8 total).
            k, jc = divmod(j, 4)
            mt8 = st[b]["mt8"]
            expq = st[b]["expq"]
            if j % 2 == 0:
                st[b]["stage"] = stgp.tile([P, 2048], F8,
                                           name=f"stage_{b}_{j}", tag="stage")
            stage = st[b]["stage"]
            pis = []
            for h in range(2):
                j0 = jc * 1024 + h * 512
                pi = kp.tile([P, 512], F32, name=f"pi_{b}_{j}_{h}", tag="big")
                nc.tensor.matmul(
                    pi,
                    lhsT=mt8[:, :, k * P:(k + 1) * P],
                    rhs=expq[:, :, j0:j0 + 512],
                    start=True, stop=True, perf_mode=DR,
                )
                pis.append(pi)
            for h in range(2):
                s0 = (j % 2) * 1024 + h * 512
                sl = stage[:, s0:s0 + 512]
                if evict[h] == "scalar":
                    nc.scalar.activation(sl, pis[h], AF.Copy, scale=ISMT * SO)
                else:
                    nc.vector.tensor_scalar(sl, pis[h], ISMT * SO, None,
                                            OP.mult)
            if j % 2 == 1:
                c0 = (jc - 1) * 1024
                nc.sync.dma_start(
                    out=out_d[b][k * P:(k + 1) * P, c0:c0 + 2048],
                    in_=stage)

        # ---- schedule ----
        # Q0 while x_kv streams in
        for j in range(8):
            emit_q_chunk(0, j)
        # branch-0 K + ctx (ctx lagging three pairs for exp/DMA slack)
        for tp in range(NTP):
            emit_k_pair(0, tp)
            if tp >= 3:
                emit_ctx(0, tp - 3)
        for tp in (NTP - 3, NTP - 2, NTP - 1):
            emit_ctx(0, tp)
        emit_pctx_evict(0)
        emit_ctxT_A(0)
        emit_mt_final(0)
        # branch-1 Q + branch-0 inter + branch-1 K/ctx interleaved
        for j in range(8):
            emit_q_chunk(1, j)
            emit_inter_chunk(0, j)
            for tp in (2 * j, 2 * j + 1):
                emit_k_pair(1, tp)
                if tp >= 3:
                    emit_ctx(1, tp - 3)
        for tp in (NTP - 3, NTP - 2, NTP - 1):
            emit_ctx(1, tp)
        emit_pctx_evict(1)
        emit_ctxT_A(1)
        emit_mt_final(1)
        # branch-1 inter tail; evictions split across scalar and vector
        for j in range(8):
            emit_inter_chunk(1, j, evict=("vector", "scalar"))

    nc.finalize()
    return nc


def _get_nc():
    if "nc" not in _CACHE:
        _CACHE["nc"] = _build()
    return _CACHE["nc"]


def _dr(x):
    # [256, n] -> DoubleRow interleave [128, 2, n]: slot s holds channel p+128s
    return np.ascontiguousarray(x.reshape(2, P, -1).transpose(1, 0, 2))


def _pack_host(inputs):
    import ml_dtypes
    f8 = ml_dtypes.float8_e4m3
    bf16 = ml_dtypes.bfloat16
    f32 = np.float32

    wks = []
    wqs = []
    a_chunks = []
    for bi, b in enumerate(("1", "2")):
        wk = np.asarray(inputs[f"w_k{b}"], f32).T * SW
        wks.append(_dr(wk))
        wqs.append(_dr(np.asarray(inputs[f"w_q{b}"], f32).T * SW))
        wv = np.asarray(inputs[f"w_v{b}"], f32)        # [256 e, 256 c]
        wpT = np.asarray(inputs[f"w_proj{b}"], f32).T  # [256 e, 256 oc]
        for p in range(2):
            for hh in range(2):
                rows = slice(p * P + hh * HD, p * P + (hh + 1) * HD)
                A = (SW * SA) * (wv[rows, :].T @ wpT[rows, :])  # [256, 256]
                a_chunks.append(A[0:P, :])
                a_chunks.append(A[P:C, :])
    wk8 = np.concatenate(wks, axis=2).astype(f8)        # [128, 2, 512]
    wq8 = np.concatenate(wqs, axis=2).astype(f8)        # [128, 2, 512]
    a8 = np.stack(a_chunks, axis=1).astype(f8)          # [128, 16, 256]
    id128 = np.eye(P, dtype=bf16)
    return (np.ascontiguousarray(wk8), np.ascontiguousarray(wq8),
            np.ascontiguousarray(a8), np.ascontiguousarray(id128))


def _xvt(x8):
    # [C, N] (already /SW) -> [128 n, 2 slot, 16 blk, 257] with ones col
    import ml_dtypes
    f8 = ml_dtypes.float8_e4m3
    t = x8.reshape(C, NTP, 2, P).transpose(3, 2, 1, 0)  # [n, s, blk, c]
    out = np.empty((P, 2, NTP, 257), dtype=f8)
    out[:, :, :, 0:C] = t.astype(f8)
    out[:, :, :, C] = np.float32(1.0)
    return np.ascontiguousarray(out)


def kernel(rgb_low, rgb_high, dsm_low, dsm_high,
           w_q1, b_q1, w_k1, b_k1, w_v1, b_v1,
           w_q2, b_q2, w_k2, b_k2, w_v2, b_v2,
           w_proj1, b_proj1, w_proj2, b_proj2, gamma, beta,
           _trace=False):
    import ml_dtypes
    from concourse.bass_utils import run_bass_kernel_spmd
    f8 = ml_dtypes.float8_e4m3
    f32 = np.float32

    # b_v is folded into the host-side A matrices only for b_v == 0 (true
    # for this problem's inputs); a nonzero b_v would need a den rank-1
    # term that this kernel does not emit.
    assert np.abs(np.asarray(b_v1)).max() == 0.0
    assert np.abs(np.asarray(b_v2)).max() == 0.0

    inputs = dict(w_q1=w_q1, w_k1=w_k1, w_v1=w_v1, w_proj1=w_proj1,
                  w_q2=w_q2, w_k2=w_k2, w_v2=w_v2, w_proj2=w_proj2)
    rl = np.asarray(rgb_low, dtype=f32)
    rh = np.asarray(rgb_high, dtype=f32)
    dl = np.asarray(dsm_low, dtype=f32)
    dh = np.asarray(dsm_high, dtype=f32)
    B = rl.shape[0]
    assert B == NCORES, f"expected batch {NCORES}, got {B}"

    wk8, wq8, a8, id128 = _pack_host(inputs)

    xq = [rl.reshape(B, C, N), rh.reshape(B, C, N)]
    xkv = [dh.reshape(B, C, N), dl.reshape(B, C, N)]

    in_maps = []
    for i in range(NCORES):
        m = {"wk8": wk8, "wq8": wq8, "a8": a8, "id128": id128}
        for b in range(2):
            x8 = xkv[b][i] / SW
            m[f"xq8_{b + 1}"] = _dr(xq[b][i] / SW).astype(f8)
            m[f"xkv8_{b + 1}"] = _dr(x8).astype(f8)
            m[f"xvt8_{b + 1}"] = _xvt(x8)
        in_maps.append(m)

    res = run_bass_kernel_spmd(nc := _get_nc(), in_maps,
                               core_ids=list(range(NCORES)), trace=_trace)

    # host: residual + b_proj + exact training-mode BN over the batch
    g = np.asarray(gamma, f32)
    be = np.asarray(beta, f32)
    bprj = [np.asarray(b_proj1, f32), np.asarray(b_proj2, f32)]
    outs = []
    for b, name in ((0, "out1"), (1, "out2")):
        inter = np.stack([np.asarray(res.results[i][name], f32)
                          for i in range(NCORES)]) / SO     # [B, C, N]
        y = xq[b] + inter + bprj[b][None, :, None]
        mu = y.mean(axis=(0, 2))
        sd = np.sqrt(y.var(axis=(0, 2)) + EPS)
        s2 = g / sd
        t2 = be - mu * s2
        outs.append((y * s2[None, :, None] + t2[None, :, None])
                    .reshape(B, C, 64, 64).astype(f32))
    if _trace:
        _CACHE["last_results"] = res
    return (outs[0], outs[1], np.asarray(dsm_low), np.asarray(dsm_high))


# revision 32
# speedup vs baseline: 1.0733x; 1.0371x over previous
# Trainium2 Bass kernel for nn_CrossFrequencyInteraction — v4.
#
# Reference computation (per batch item, two symmetric branches):
#   q = Wq @ x_q;  k = Wk @ x_kv;  v = Wv @ x_kv          (1x1 convs, C=256)
#   out = softmax_n(q) applied against ctx = softmax_n(k) @ v^T  (linear attn)
#   inter = Wp @ out;  x_q += inter
#   then training-mode BatchNorm over (B,H,W) on both updated rgb tensors.
#
# Sharding: data-parallel over batch (B=8 -> 1 item per core, 8 cores).
#
# v4 structure:
#   - Device computes only `inter` per branch (bf16 out); host adds the
#     residual + b_proj and applies exact batch BN stats in f32.
#   - THE V CONV AND ITS PSUM->SBUF CAST ARE GONE: since the attention is
#     linear, ctx·Wp factorizes as ctx_raw·A with A_h = 8·SA·Wv_h^T·Wp_h
#     precomputed on host per head.  The device computes
#     ctx_raw[d, c] = sum_n exp(k)[n, d]·x_kv[c, n]/8 by streaming a
#     host-uploaded TRANSPOSED x_kv (fp8, ones column appended for the
#     softmax-k denominator).  The ctx stream is DMA-fed, so the vector
#     engine is off the PE's critical loop entirely.
#   - b_q, b_k shift softmax inputs per-row -> cancel exactly.
#     b_v assumed zero (asserted; true for this problem's inputs).
#     b_proj added by the host.
#   - M^T assembly: PE-transpose ctx_raw chunks, two accumulating
#     A-matmuls per head (concurrent column-groups via tile_position),
#     softmax denominators folded into the per-row fac scale; the inter
#     phase is a single fp8 DR matmul stream per chunk, descale ISMT at
#     eviction.
#
# Schedule: Q0 first (while x_kv streams in), then KV0+ctx0, M^T(0),
# then {Q1, inter0, KV1+ctx1} interleaved to keep the PE dense, M^T(1),
# inter1 tail with evictions split scalar/vector.

import numpy as np

C = 256
N = 4096
P = 128
NTP = 16          # pairs of 128-wide n-tiles (KV phase)
NCORES = 8
HD = 64
EPS = 1e-5
SW = 8.0          # host weight/input scale around fp8
SA = 64.0         # extra host scale on the A matrices (fp8 normal range)
SMT = float(2.0 ** 22)   # fp8 scale for M^T
ISMT = float(2.0 ** -22)
SO = float(2.0 ** 12)    # output scale: inter*SO sits in fp8 normal range

_CACHE = {}


def _build():
    import concourse.bass as bass
    import concourse.bacc as bacc
    import concourse.tile as tile
    from concourse import mybir
    from contextlib import ExitStack

    F32 = mybir.dt.float32
    BF16 = mybir.dt.bfloat16
    F8 = mybir.dt.float8e4
    OP = mybir.AluOpType
    AF = mybir.ActivationFunctionType
    AX = mybir.AxisListType
    DR = mybir.MatmulPerfMode.DoubleRow

    nc = bacc.Bacc("TRN2", num_devices=NCORES)

    xq8_d = [nc.dram_tensor(n_, [P, 2, N], F8, kind="ExternalInput")
             for n_ in ("xq8_1", "xq8_2")]
    xkv8_d = [nc.dram_tensor(n_, [P, 2, N], F8, kind="ExternalInput")
              for n_ in ("xkv8_1", "xkv8_2")]
    # xvt8: transposed x_kv/8 with ones col: [n 128, slot 2, blk 16, 257]
    xvt8_d = [nc.dram_tensor(n_, [P, 2, NTP, 257], F8, kind="ExternalInput")
              for n_ in ("xvt8_1", "xvt8_2")]
    # wk8: [128,2,512]: per branch 256 cols of Wk^T*8, DR-interleaved
    wk8_d = nc.dram_tensor("wk8", [P, 2, 512], F8, kind="ExternalInput")
    # wq8: [128,2,512]: per branch 256 cols of Wq^T*8, DR-interleaved
    wq8_d = nc.dram_tensor("wq8", [P, 2, 512], F8, kind="ExternalInput")
    # a8: [128, 16, 256]: A_h chunks, idx = ((b*2+p)*2+hh)*2+q
    a8_d = nc.dram_tensor("a8", [P, 16, 256], F8, kind="ExternalInput")
    id_d = nc.dram_tensor("id128", [P, P], BF16, kind="ExternalInput")
    # inter output, fp8 scaled by SO (host descales, adds residual + BN)
    out_d = [nc.dram_tensor(n_, [C, N], F8, kind="ExternalOutput")
             for n_ in ("out1", "out2")]

    with ExitStack() as ctx:
        tc = ctx.enter_context(tile.TileContext(nc))
        const = ctx.enter_context(tc.tile_pool(name="const", bufs=1))
        xp = ctx.enter_context(tc.tile_pool(name="xp", bufs=1))
        eqp = ctx.enter_context(tc.tile_pool(name="eqp", bufs=2))
        ekp = ctx.enter_context(tc.tile_pool(name="ekp", bufs=4))
        misc = ctx.enter_context(tc.tile_pool(name="misc", bufs=2))
        stgp = ctx.enter_context(tc.tile_pool(name="stgp", bufs=4))
        # 1-bank psum tiles: K-conv pair tiles, inter chunk halves, and the
        # M^T-build accumulators (which run between K/inter allocations);
        # folding the latter into this ring frees a bank for a 4-deep ring
        # and lets the two head-pairs' M^T builds overlap.
        kp = ctx.enter_context(tc.tile_pool(name="kp", bufs=4, space="PSUM"))
        qp = ctx.enter_context(tc.tile_pool(name="qp", bufs=1, space="PSUM"))
        ctxp = ctx.enter_context(tc.tile_pool(name="ctxp", bufs=1,
                                              space="PSUM"))

        wk8 = const.tile([P, 2, 512], F8, name="wk8", tag="wk8")
        wq8 = const.tile([P, 2, 512], F8, name="wq8", tag="wq8")
        a8 = const.tile([P, 16, 256], F8, name="a8", tag="a8")
        id_sb = const.tile([P, P], BF16, name="id", tag="id")
        xkv8 = [xp.tile([P, 2, N], F8, name=f"xkv8_{b}", tag=f"xkv8_{b}")
                for b in range(2)]
        xq8 = [xp.tile([P, 2, N], F8, name=f"xq8_{b}", tag=f"xq8_{b}")
               for b in range(2)]
        xvt8 = [xp.tile([P, 2, NTP, 257], F8, name=f"xvt8_{b}",
                        tag=f"xvt8_{b}") for b in range(2)]

        # Few, large DMA issues ordered by first consumption; a small first
        # chunk unblocks the PE as early as possible.
        nc.sync.dma_start(out=wq8, in_=wq8_d[:, :, :])
        nc.sync.dma_start(out=xq8[0][:, :, 0:1024], in_=xq8_d[0][:, :, 0:1024])
        nc.sync.dma_start(out=xq8[0][:, :, 1024:N], in_=xq8_d[0][:, :, 1024:N])
        nc.sync.dma_start(out=wk8, in_=wk8_d[:, :, :])
        nc.sync.dma_start(out=xkv8[0][:, :, 0:2048],
                          in_=xkv8_d[0][:, :, 0:2048])
        nc.sync.dma_start(out=xvt8[0][:, :, 0:8, :],
                          in_=xvt8_d[0][:, :, 0:8, :])
        nc.sync.dma_start(out=xkv8[0][:, :, 2048:N],
                          in_=xkv8_d[0][:, :, 2048:N])
        nc.sync.dma_start(out=xvt8[0][:, :, 8:NTP, :],
                          in_=xvt8_d[0][:, :, 8:NTP, :])
        nc.sync.dma_start(out=xkv8[1], in_=xkv8_d[1][:, :, :])
        nc.sync.dma_start(out=xvt8[1], in_=xvt8_d[1][:, :, :, :])
        nc.sync.dma_start(out=xq8[1], in_=xq8_d[1][:, :, :])
        nc.scalar.dma_start(out=a8, in_=a8_d[:, :, :])
        nc.scalar.dma_start(out=id_sb, in_=id_d[:, :])

        st = {0: {}, 1: {}}

        # ---- emission helpers (issue order == engine execution order) ----

        def emit_k_pair(b, tp):
            # K conv for n-tiles (2tp, 2tp+1) into a 1-bank pair tile
            # [P, tt, 256] + exp -> ek (fp8)
            w0 = b * 256
            pr = kp.tile([P, 2, 256], F32, name=f"pr_{b}_{tp}", tag="big")
            for tt in range(2):
                s128 = slice(tp * 256 + tt * P, tp * 256 + (tt + 1) * P)
                nc.tensor.matmul(
                    pr[:, tt, :],
                    lhsT=xkv8[b][:, :, s128],
                    rhs=wk8[:, :, w0:w0 + 256],
                    start=True, stop=True, perf_mode=DR,
                )
            ek = ekp.tile([P, 2, 256], F8, name=f"ek_{b}_{tp}", tag="ek")
            nc.scalar.activation(ek, pr, AF.Exp)
            st[b][f"ek{tp}"] = ek

        def emit_ctx(b, tp):
            # fp8-DR ctx_raw matmuls for pair tp (issued two pairs behind):
            # [d 128, c 256 + den] accumulated over n; rhs is the DMA-fed
            # transposed x_kv tile.
            pctx = st[b].get("pctx")
            if pctx is None:
                pctx = ctxp.tile([P, 2, 257], F32, name=f"pctx_{b}",
                                 tag="pctx")
                st[b]["pctx"] = pctx
            ek = st[b].pop(f"ek{tp}")
            for p in range(2):
                nc.tensor.matmul(
                    pctx[:, p, :],
                    lhsT=ek[:, :, p * P:(p + 1) * P],
                    rhs=xvt8[b][:, :, tp, :],
                    start=(tp == 0), stop=(tp == NTP - 1),
                    perf_mode=DR, skip_group_check=True,
                )

        def emit_q_chunk(b, j):
            # Q conv (fp8-DR) + exp for a [128, 1024] chunk, both k-blocks
            # interleaved as j = k*4 + jc
            k, jc = divmod(j, 4)
            if j == 0:
                st[b]["sqp"] = misc.tile([P, 2, 4], F32, name=f"sqp_{b}",
                                         tag="sqp")
                st[b]["expq"] = eqp.tile([P, 2, N], F8, name=f"expq_{b}",
                                         tag="expq")
            wk = slice(b * 256 + k * P, b * 256 + (k + 1) * P)
            pq = qp.tile([P, 1024], F32, name=f"pq_{b}_{j}", tag="pq")
            for h in range(2):
                s = slice(jc * 1024 + h * 512, jc * 1024 + (h + 1) * 512)
                nc.tensor.matmul(
                    pq[:, h * 512:(h + 1) * 512],
                    lhsT=wq8[:, :, wk],
                    rhs=xq8[b][:, :, s],
                    start=True, stop=True, perf_mode=DR,
                )
            nc.scalar.activation(
                st[b]["expq"][:, k, jc * 1024:(jc + 1) * 1024], pq, AF.Exp,
                accum_out=st[b]["sqp"][:, k, jc:jc + 1])

        def emit_pctx_evict(b):
            # free the pctx banks for the other branch; den + raw ctx
            pctx = st[b].pop("pctx")
            denT = misc.tile([P, 2], F32, name=f"denT_{b}", tag="denT")
            for p in range(2):
                nc.vector.tensor_copy(denT[:, p:p + 1], pctx[:, p, 256:257])
            ctxS = misc.tile([P, 2, 256], BF16, name=f"ctxS_{b}", tag="ctxS")
            nc.vector.tensor_copy(ctxS, pctx[:, :, 0:256])
            st[b]["denT"] = denT
            st[b]["ctxS"] = ctxS

        def emit_ctxT_A(b):
            # PE transpose of ctx_raw chunks + accumulating A-matmuls:
            # pmt[64hh:64hh+64, :] = sum_q ctx_rawT_chunk_q^T · A8 chunk.
            # The two heads of a pair run in concurrent PE column groups
            # (tile_position).  pmt is evicted to sbuf bf16 so the single
            # tiny bank is reused p=0 then p=1.
            ctxS = st[b]["ctxS"]
            pmtb = misc.tile([P, 2, 256], BF16, name=f"pmtb_{b}", tag="pmtb")
            for p in range(2):
                tiny = kp.tile([P, 384], F32, name=f"tiny_{b}_{p}",
                               tag="big")
                pmt = tiny[:, 128:384]
                for hh in range(2):
                    s64 = slice(hh * HD, (hh + 1) * HD)
                    ptr = tiny[:, 0:64].bitcast(BF16)
                    for q in range(2):
                        nc.tensor.transpose(
                            ptr[:, HD * q:HD * (q + 1)],
                            ctxS[s64, p, 128 * q:128 * (q + 1)],
                            id_sb[s64, s64])
                    ptrS = misc.tile([P, 2 * HD], BF16,
                                     name=f"ptrS_{b}_{p}_{hh}", tag="ptrS")
                    nc.vector.tensor_copy(ptrS, ptr)
                    for q in range(2):
                        idx = ((b * 2 + p) * 2 + hh) * 2 + q
                        nc.tensor.matmul(
                            pmt[s64, :],
                            lhsT=ptrS[:, HD * q:HD * (q + 1)],
                            rhs=a8[:, idx, :],
                            start=(q == 0), stop=(q == 1),
                            skip_group_check=True,
                            tile_position=(0, hh * HD),
                        )
                nc.vector.tensor_copy(pmtb[:, p, :], pmt)
            st[b]["pmtb"] = pmtb

        def emit_mt_final(b):
            # fac = 2^22 / (SA * den_k * sum_q) per d-row; mt8 = pmtb * fac
            st[b]["mt8"] = misc.tile([P, 2, 256], F8, name=f"mt8_{b}",
                                     tag="mt8")
            for p_ in range(2):
                sq2 = misc.tile([P, 1], F32, name=f"sq2_{b}_{p_}", tag="sq2")
                nc.vector.reduce_sum(sq2, st[b]["sqp"][:, p_, :], axis=AX.X)
                fde = misc.tile([P, 1], F32, name=f"fde_{b}_{p_}", tag="fde")
                nc.vector.scalar_tensor_tensor(
                    fde, st[b]["denT"][:, p_:p_ + 1], ISMT * SA, sq2,
                    OP.mult, OP.mult)
                fac = misc.tile([P, 1], F32, name=f"fac_{b}_{p_}", tag="fac")
                nc.vector.reciprocal(fac, fde)
                nc.vector.tensor_scalar(st[b]["mt8"][:, p_, :],
                                        st[b]["pmtb"][:, p_, :], fac, None,
                                        OP.mult)

        def emit_inter_chunk(b, j, evict=("vector", "vector")):
            # inter matmuls into two 1-bank psum halves + descale eviction
            # into a [128, 2048] fp8 stage shared by two consecutive j
            # chunks; one output DMA per stage (8 total).
            k, jc = divmod(j, 4)
            mt8 = st[b]["mt8"]
            expq = st[b]["expq"]
            if j % 2 == 0:
                st[b]["stage"] = stgp.tile([P, 2048], F8,
                                           name=f"stage_{b}_{j}", tag="stage")
            stage = st[b]["stage"]
            pis = []
            for h in range(2):
                j0 = jc * 1024 + h * 512
                pi = kp.tile([P, 512], F32, name=f"pi_{b}_{j}_{h}", tag="big")
                nc.tensor.matmul(
                    pi,
                    lhsT=mt8[:, :, k * P:(k + 1) * P],
                    rhs=expq[:, :, j0:j0 + 512],
                    start=True, stop=True, perf_mode=DR,
                )
                pis.append(pi)
            for h in range(2):
                s0 = (j % 2) * 1024 + h * 512
                sl = stage[:, s0:s0 + 512]
                if evict[h] == "scalar":
                    nc.scalar.activation(sl, pis[h], AF.Copy, scale=ISMT * SO)
                else:
                    nc.vector.tensor_scalar(sl, pis[h], ISMT * SO, None,
                                            OP.mult)
            if j % 2 == 1:
                c0 = (jc - 1) * 1024
                nc.sync.dma_start(
                    out=out_d[b][k * P:(k + 1) * P, c0:c0 + 2048],
                    in_=stage)

        # ---- schedule ----
        # Q0 while x_kv streams in
        for j in range(8):
            emit_q_chunk(0, j)
        # branch-0 K + ctx (ctx lagging three pairs for exp/DMA slack)
        for tp in range(NTP):
            emit_k_pair(0, tp)
            if tp >= 3:
                emit_ctx(0, tp - 3)
        for tp in (NTP - 3, NTP - 2, NTP - 1):
            emit_ctx(0, tp)
        emit_pctx_evict(0)
        emit_ctxT_A(0)
        emit_mt_final(0)
        # branch-1 Q + branch-0 inter + branch-1 K/ctx interleaved
        for j in range(8):
            emit_q_chunk(1, j)
            emit_inter_chunk(0, j)
            for tp in (2 * j, 2 * j + 1):
                emit_k_pair(1, tp)
                if tp >= 3:
                    emit_ctx(1, tp - 3)
        for tp in (NTP - 3, NTP - 2, NTP - 1):
            emit_ctx(1, tp)
        emit_pctx_evict(1)
        emit_ctxT_A(1)
        emit_mt_final(1)
        # branch-1 inter tail; evictions split across scalar and vector
        for j in range(8):
            emit_inter_chunk(1, j, evict=("vector", "scalar"))

    nc.finalize()
    return nc


def _get_nc():
    if "nc" not in _CACHE:
        _CACHE["nc"] = _build()
    return _CACHE["nc"]


def _dr(x):
    # [256, n] -> DoubleRow interleave [128, 2, n]: slot s holds channel p+128s
    return np.ascontiguousarray(x.reshape(2, P, -1).transpose(1, 0, 2))


def _pack_host(inputs):
    import ml_dtypes
    f8 = ml_dtypes.float8_e4m3
    bf16 = ml_dtypes.bfloat16
    f32 = np.float32

    wks = []
    wqs = []
    a_chunks = []
    for bi, b in enumerate(("1", "2")):
        wk = np.asarray(inputs[f"w_k{b}"], f32).T * SW
        wks.append(_dr(wk))
        wqs.append(_dr(np.asarray(inputs[f"w_q{b}"], f32).T * SW))
        wv = np.asarray(inputs[f"w_v{b}"], f32)        # [256 e, 256 c]
        wpT = np.asarray(inputs[f"w_proj{b}"], f32).T  # [256 e, 256 oc]
        for p in range(2):
            for hh in range(2):
                rows = slice(p * P + hh * HD, p * P + (hh + 1) * HD)
                A = (SW * SA) * (wv[rows, :].T @ wpT[rows, :])  # [256, 256]
                a_chunks.append(A[0:P, :])
                a_chunks.append(A[P:C, :])
    wk8 = np.concatenate(wks, axis=2).astype(f8)        # [128, 2, 512]
    wq8 = np.concatenate(wqs, axis=2).astype(f8)        # [128, 2, 512]
    a8 = np.stack(a_chunks, axis=1).astype(f8)          # [128, 16, 256]
    id128 = np.eye(P, dtype=bf16)
    return (np.ascontiguousarray(wk8), np.ascontiguousarray(wq8),
            np.ascontiguousarray(a8), np.ascontiguousarray(id128))


def _xvt(x8):
    # [C, N] (already /SW) -> [128 n, 2 slot, 16 blk, 257] with ones col
    import ml_dtypes
    f8 = ml_dtypes.float8_e4m3
    t = x8.reshape(C, NTP, 2, P).transpose(3, 2, 1, 0)  # [n, s, blk, c]
    out = np.empty((P, 2, NTP, 257), dtype=f8)
    out[:, :, :, 0:C] = t.astype(f8)
    out[:, :, :, C] = np.float32(1.0)
    return np.ascontiguousarray(out)


def kernel(rgb_low, rgb_high, dsm_low, dsm_high,
           w_q1, b_q1, w_k1, b_k1, w_v1, b_v1,
           w_q2, b_q2, w_k2, b_k2, w_v2, b_v2,
           w_proj1, b_proj1, w_proj2, b_proj2, gamma, beta,
           _trace=False):
    import ml_dtypes
    from concourse.bass_utils import run_bass_kernel_spmd
    f8 = ml_dtypes.float8_e4m3
    f32 = np.float32

    # b_v is folded into the host-side A matrices only for b_v == 0 (true
    # for this problem's inputs); a nonzero b_v would need a den rank-1
    # term that this kernel does not emit.
    assert np.abs(np.asarray(b_v1)).max() == 0.0
    assert np.abs(np.asarray(b_v2)).max() == 0.0

    inputs = dict(w_q1=w_q1, w_k1=w_k1, w_v1=w_v1, w_proj1=w_proj1,
                  w_q2=w_q2, w_k2=w_k2, w_v2=w_v2, w_proj2=w_proj2)
    rl = np.asarray(rgb_low, dtype=f32)
    rh = np.asarray(rgb_high, dtype=f32)
    dl = np.asarray(dsm_low, dtype=f32)
    dh = np.asarray(dsm_high, dtype=f32)
    B = rl.shape[0]
    assert B == NCORES, f"expected batch {NCORES}, got {B}"

    wk8, wq8, a8, id128 = _pack_host(inputs)

    xq = [rl.reshape(B, C, N), rh.reshape(B, C, N)]
    xkv = [dh.reshape(B, C, N), dl.reshape(B, C, N)]

    in_maps = []
    for i in range(NCORES):
        m = {"wk8": wk8, "wq8": wq8, "a8": a8, "id128": id128}
        for b in range(2):
            x8 = xkv[b][i] / SW
            m[f"xq8_{b + 1}"] = _dr(xq[b][i] / SW).astype(f8)
            m[f"xkv8_{b + 1}"] = _dr(x8).astype(f8)
            m[f"xvt8_{b + 1}"] = _xvt(x8)
        in_maps.append(m)

    res = run_bass_kernel_spmd(nc := _get_nc(), in_maps,
                               core_ids=list(range(NCORES)), trace=_trace)

    # host: residual + b_proj + exact training-mode BN over the batch
    g = np.asarray(gamma, f32)
    be = np.asarray(beta, f32)
    bprj = [np.asarray(b_proj1, f32), np.asarray(b_proj2, f32)]
    outs = []
    for b, name in ((0, "out1"), (1, "out2")):
        inter = np.stack([np.asarray(res.results[i][name], f32)
                          for i in range(NCORES)]) / SO     # [B, C, N]
        y = xq[b] + inter + bprj[b][None, :, None]
        mu = y.mean(axis=(0, 2))
        sd = np.sqrt(y.var(axis=(0, 2)) + EPS)
        s2 = g / sd
        t2 = be - mu * s2
        outs.append((y * s2[None, :, None] + t2[None, :, None])
                    .reshape(B, C, 64, 64).astype(f32))
    if _trace:
        _CACHE["last_results"] = res
    return (outs[0], outs[1], np.asarray(dsm_low), np.asarray(dsm_high))
